# revision 1
# baseline (speedup 1.0000x reference)
"""Sliding-window causal self-attention (B=2, T=2048, C=1024, H=16, Dh=64,
window=256) + QKV/out projections, sharded over 8 NeuronCores as
data-parallel over B (2) x tensor-parallel over head groups (4 heads/core).

Per-core layout trick: everything on the "token-transposed" side
(features on partitions, tokens on free axis) so the QK^T, P@V and output
projection all contract on the partition axis with zero device transposes
of activations; the only transpose (P -> P^T) is fused with the softmax
denominator normalization as a matmul against diag(1/rowsum).
"""

import math

import numpy as np

B = 2
T = 2048
C = 1024
H = 16
DH = 64
WINDOW = 256
G = H // 4 // 1  # head groups (4 heads per group) -> 4 groups
HEADS_PER_CORE = 4
N_CORES = 8
QT = T // 128  # 16 query tiles of 128
FQ = HEADS_PER_CORE * DH  # 256 local features

_PROGRAM = None  # (nc, input_names) cache — compile once per process


def _emit(nc, tc, aps, ctx):
    from concourse import mybir

    f32 = mybir.dt.float32
    bf16 = mybir.dt.bfloat16
    Exp = mybir.ActivationFunctionType.Exp

    xT, wT, woT, cos4, sin4, amask, ident, y = (
        aps["xT"], aps["wT"], aps["woT"], aps["cos4"], aps["sin4"],
        aps["amask"], aps["ident"], aps["y"],
    )

    consts = ctx.enter_context(tc.tile_pool(name="consts", bufs=1))
    stage = ctx.enter_context(tc.tile_pool(name="stage", bufs=1))
    work = ctx.enter_context(tc.tile_pool(name="work", bufs=3))
    tmp = ctx.enter_context(tc.tile_pool(name="tmp", bufs=4))
    small = ctx.enter_context(tc.tile_pool(name="small", bufs=4))
    ysbp = ctx.enter_context(tc.tile_pool(name="ysbp", bufs=2))
    pmm = ctx.enter_context(tc.tile_pool(name="pmm", bufs=2, space="PSUM"))
    ps = ctx.enter_context(tc.tile_pool(name="ps", bufs=2, space="PSUM"))
    ppt = ctx.enter_context(tc.tile_pool(name="ppt", bufs=2, space="PSUM"))
    po = ctx.enter_context(tc.tile_pool(name="po", bufs=2, space="PSUM"))

    # ---- resident inputs ----
    xT_sb = consts.tile([128, 8 * T], bf16, tag="xT")  # [C-part, (kc t)]
    nc.sync.dma_start(
        out=xT_sb.rearrange("p (kc t) -> p kc t", kc=8),
        in_=xT.rearrange("(kc p) t -> p kc t", p=128),
    )
    wT_sb = consts.tile([128, 8 * 768], bf16, tag="wT")
    nc.sync.dma_start(
        out=wT_sb.rearrange("p (kc f) -> p kc f", kc=8),
        in_=wT.rearrange("(kc p) f -> p kc f", p=128),
    )
    woT_sb = consts.tile([128, 2 * C], bf16, tag="woT")
    nc.sync.dma_start(
        out=woT_sb.rearrange("p (kc e) -> p kc e", kc=2),
        in_=woT.rearrange("(kc p) e -> p kc e", p=128),
    )
    cos_sb = consts.tile([128, T], bf16, tag="cos")
    nc.sync.dma_start(out=cos_sb, in_=cos4)
    sin_sb = consts.tile([128, T], bf16, tag="sin")
    nc.sync.dma_start(out=sin_sb, in_=sin4)
    # additive band mask for PSUM preload: [strict-upper | lower-incl]
    amask_sb = consts.tile([128, 256], bf16, tag="amask")
    nc.sync.dma_start(out=amask_sb, in_=amask)
    id_sb = consts.tile([128, 128], bf16, tag="ident")
    nc.sync.dma_start(out=id_sb, in_=ident)

    # ---- persistent intermediates ----
    # pre-RoPE blocks [q_x1, q_x2, k_x1, k_x2], each [128=(4h x 32d), T]
    pre = [stage.tile([128, T], bf16, tag=f"pre{i}", name=f"pre{i}")
           for i in range(4)]
    rot = [stage.tile([128, T], bf16, tag=f"rot{i}", name=f"rot{i}")
           for i in range(4)]
    qhT = stage.tile([64, HEADS_PER_CORE * T], bf16, tag="qhT")
    khT = stage.tile([64, HEADS_PER_CORE * T], bf16, tag="khT")
    v_sb = stage.tile([128, QT * FQ], bf16, tag="v")  # [tk-part, (tt f)]
    attnT = stage.tile([128, 2 * T], bf16, tag="attnT")  # [(f%128), (kc t)]

    # ---- phase 1: QKV projection ----
    for split in range(4):  # 512-token slices
        tsl = slice(split * 512, (split + 1) * 512)
        for blk in range(4):  # q_x1 q_x2 k_x1 k_x2
            acc = pmm.tile([128, 512], f32, tag="mm")
            for kc in range(8):
                nc.tensor.matmul(
                    acc,
                    lhsT=wT_sb[:, kc * 768 + blk * 128:kc * 768 + (blk + 1) * 128],
                    rhs=xT_sb[:, kc * T + split * 512:kc * T + (split + 1) * 512],
                    start=(kc == 0),
                    stop=(kc == 7),
                )
            nc.any.tensor_copy(pre[blk][:, tsl], acc)
        # RoPE on this token slice: rot1 = x1*cos - x2*sin ; rot2 = x2*cos + x1*sin
        for pair in range(2):  # 0 -> q, 1 -> k
            x1, x2 = pre[2 * pair][:, tsl], pre[2 * pair + 1][:, tsl]
            r1, r2 = rot[2 * pair][:, tsl], rot[2 * pair + 1][:, tsl]
            t1 = tmp.tile([128, 512], bf16, tag="t1")
            t2 = tmp.tile([128, 512], bf16, tag="t2")
            t3 = tmp.tile([128, 512], bf16, tag="t3")
            t4 = tmp.tile([128, 512], bf16, tag="t4")
            nc.vector.tensor_mul(t1, x1, cos_sb[:, tsl])
            nc.vector.tensor_mul(t2, x2, sin_sb[:, tsl])
            nc.vector.tensor_sub(r1, t1, t2)
            nc.vector.tensor_mul(t3, x2, cos_sb[:, tsl])
            nc.vector.tensor_mul(t4, x1, sin_sb[:, tsl])
            nc.vector.tensor_add(r2, t3, t4)

    # repack rotated q/k into head-contiguous [64, (h t)] via SBUF->SBUF DMA
    for hl in range(HEADS_PER_CORE):
        for half in range(2):
            nc.sync.dma_start(
                out=qhT[half * 32:(half + 1) * 32, hl * T:(hl + 1) * T],
                in_=rot[half][hl * 32:(hl + 1) * 32, :],
            )
            nc.sync.dma_start(
                out=khT[half * 32:(half + 1) * 32, hl * T:(hl + 1) * T],
                in_=rot[2 + half][hl * 32:(hl + 1) * 32, :],
            )

    # ---- phase 2: v tiles + banded attention + out-proj, interleaved ----
    for qt in range(QT):
        # v tile qt in natural (token-partition) layout [128, 256]
        acc = pmm.tile([128, FQ], f32, tag="mm")
        for kc in range(8):
            nc.tensor.matmul(
                acc,
                lhsT=xT_sb[:, kc * T + qt * 128:kc * T + (qt + 1) * 128],
                rhs=wT_sb[:, kc * 768 + 512:kc * 768 + 768],
                start=(kc == 0),
                stop=(kc == 7),
            )
        nc.any.tensor_copy(v_sb[:, qt * FQ:(qt + 1) * FQ], acc)

        nkt = min(qt + 1, 3)  # key tiles in window
        w = 128 * nkt
        kt0 = max(qt - 2, 0)
        for hl in range(HEADS_PER_CORE):
            # scores, then the additive band mask folded in-place in PSUM:
            # global block positions ga=0 (strict-upper mask), 1 (all
            # in-window), 2 (lower-incl mask); masked entries become -60 ish
            # so exp()'s fused accum_out directly yields masked row sums.
            s = ps.tile([128, 384], f32, tag="s")
            nc.tensor.matmul(
                s[:, :w],
                lhsT=qhT[:, hl * T + qt * 128:hl * T + (qt + 1) * 128],
                rhs=khT[:, hl * T + kt0 * 128:hl * T + kt0 * 128 + w],
                start=True,
                stop=True,
            )
            if qt >= 2:  # both triangle blocks in one strided op
                sv = s.rearrange("p (b w) -> p b w", b=3)[:, 0::2, :]
                mv = amask_sb.rearrange("p (b w) -> p b w", b=2)
                nc.vector.tensor_add(sv, sv, mv)
            else:  # only the lower-incl block (last valid block)
                seg = s[:, (nkt - 1) * 128:nkt * 128]
                nc.vector.tensor_add(seg, seg, amask_sb[:, 128:256])
            p = work.tile([128, 384], bf16, tag="p")
            rs = small.tile([128, 1], f32, tag="rs")
            nc.scalar.activation(p[:, :w], s[:, :w], Exp, accum_out=rs)
            rc = small.tile([128, 1], f32, tag="rc")
            nc.vector.reciprocal(rc, rs)
            diag = work.tile([128, 128], bf16, tag="diag")
            nc.vector.tensor_scalar_mul(diag, id_sb, rc)
            # P^T with per-row normalization fused: PT[:, q] = P[q, :] / rowsum[q]
            pt = ppt.tile([128, 384], f32, tag="pt")
            for a in range(nkt):
                nc.tensor.matmul(
                    pt[:, a * 128:(a + 1) * 128],
                    lhsT=p[:, a * 128:(a + 1) * 128],
                    rhs=diag,
                    start=True,
                    stop=True,
                )
            pt_sb = work.tile([128, 384], bf16, tag="ptsb")
            nc.any.tensor_copy(pt_sb[:, :w], pt[:, :w])
            o = po.tile([64, 128], f32, tag="o")
            for a in range(nkt):
                kt = kt0 + a
                nc.tensor.matmul(
                    o,
                    lhsT=v_sb[:, kt * FQ + hl * DH:kt * FQ + (hl + 1) * DH],
                    rhs=pt_sb[:, a * 128:(a + 1) * 128],
                    start=(a == 0),
                    stop=(a == nkt - 1),
                )
            dst_r = (hl % 2) * 64
            dst_c = (hl // 2) * T + qt * 128
            nc.any.tensor_copy(attnT[dst_r:dst_r + 64, dst_c:dst_c + 128], o)

        # out-proj for token tile qt (all 4 heads of this qt are done)
        ysb = ysbp.tile([128, C], bf16, tag="ysb")
        for nh in range(2):
            acc = pmm.tile([128, 512], f32, tag="mm")
            for kc in range(2):
                nc.tensor.matmul(
                    acc,
                    lhsT=attnT[:, kc * T + qt * 128:kc * T + (qt + 1) * 128],
                    rhs=woT_sb[:, kc * C + nh * 512:kc * C + (nh + 1) * 512],
                    start=(kc == 0),
                    stop=(kc == 1),
                )
            nc.any.tensor_copy(ysb[:, nh * 512:(nh + 1) * 512], acc)
        nc.sync.dma_start(out=y[qt * 128:(qt + 1) * 128, :], in_=ysb)


def _build_program():
    import concourse.tile as tile
    from concourse import bacc, mybir

    f32 = mybir.dt.float32
    bf16 = mybir.dt.bfloat16

    nc = bacc.Bacc("TRN2", target_bir_lowering=False, debug=False,
                   num_devices=N_CORES)
    aps = {
        "xT": nc.dram_tensor("xT", [C, T], bf16, kind="ExternalInput").ap(),
        "wT": nc.dram_tensor("wT", [C, 768], bf16, kind="ExternalInput").ap(),
        "woT": nc.dram_tensor("woT", [FQ, C], bf16, kind="ExternalInput").ap(),
        "cos4": nc.dram_tensor("cos4", [128, T], bf16, kind="ExternalInput").ap(),
        "sin4": nc.dram_tensor("sin4", [128, T], bf16, kind="ExternalInput").ap(),
        "amask": nc.dram_tensor("amask", [128, 256], bf16, kind="ExternalInput").ap(),
        "ident": nc.dram_tensor("ident", [128, 128], bf16, kind="ExternalInput").ap(),
        "y": nc.dram_tensor("y", [T, C], bf16, kind="ExternalOutput").ap(),
    }
    from contextlib import ExitStack

    with tile.TileContext(nc) as tc, ExitStack() as ctx:
        _emit(nc, tc, aps, ctx)
    nc.compile()
    return nc


def _get_program():
    global _PROGRAM
    if _PROGRAM is None:
        _PROGRAM = _build_program()
    return _PROGRAM


def _host_inputs(x, w_qkv, w_out):
    import ml_dtypes

    bf16 = ml_dtypes.bfloat16
    x = np.asarray(x, np.float32)
    w_qkv = np.asarray(w_qkv, np.float32)
    w_out = np.asarray(w_out, np.float32)

    wq, wk, wv = w_qkv[0:C], w_qkv[C:2 * C], w_qkv[2 * C:3 * C]
    scale = 1.0 / math.sqrt(DH)

    # RoPE tables (transposed, tiled over the 4 heads of a block)
    inv_freq = 1.0 / (10000.0 ** (np.arange(0, DH, 2, dtype=np.float32) / DH))
    freqs = np.outer(np.arange(T, dtype=np.float32), inv_freq)  # [T, 32]
    cos4 = np.ascontiguousarray(np.tile(np.cos(freqs).T, (4, 1))).astype(bf16)
    sin4 = np.ascontiguousarray(np.tile(np.sin(freqs).T, (4, 1))).astype(bf16)

    # additive band masks [128, 256] = [strict-upper (c > i) | lower-incl
    # (c <= i)]: 0 where allowed, -60 where masked (exp -> 0 in bf16)
    i = np.arange(128)[:, None]
    c = np.arange(128)[None, :]
    m_up = np.where(c > i, 0.0, -60.0).astype(np.float32)
    m_lo = np.where(c <= i, 0.0, -60.0).astype(np.float32)
    amask = np.ascontiguousarray(
        np.concatenate([m_up, m_lo], axis=1)).astype(bf16)
    ident = np.eye(128, dtype=np.float32).astype(bf16)

    xT = [np.ascontiguousarray(x[b].T).astype(bf16) for b in range(B)]

    in_maps = []
    for core in range(N_CORES):
        b, g = divmod(core, 4)
        hs = range(4 * g, 4 * g + 4)
        rows = []
        for half in range(2):  # q_x1, q_x2
            rows.append(np.concatenate(
                [wq[h * DH + 32 * half:h * DH + 32 * half + 32] for h in hs]) * scale)
        for half in range(2):  # k_x1, k_x2
            rows.append(np.concatenate(
                [wk[h * DH + 32 * half:h * DH + 32 * half + 32] for h in hs]))
        rows.append(wv[g * FQ:(g + 1) * FQ])
        wmat = np.concatenate(rows)  # [768, C]
        wT = np.ascontiguousarray(wmat.T).astype(bf16)
        woT = np.ascontiguousarray(w_out[:, g * FQ:(g + 1) * FQ].T).astype(bf16)
        in_maps.append({
            "xT": xT[b], "wT": wT, "woT": woT,
            "cos4": cos4, "sin4": sin4, "amask": amask, "ident": ident,
        })
    return in_maps


def kernel(x, w_qkv, w_out, _trace=False):
    from concourse import bass_utils

    nc = _get_program()
    in_maps = _host_inputs(x, w_qkv, w_out)
    res = bass_utils.run_bass_kernel_spmd(
        nc, in_maps, core_ids=list(range(N_CORES)), trace=_trace,
    )
    parts = [res.results[core]["y"].astype(np.float32) for core in range(N_CORES)]
    out = np.stack([
        parts[0] + parts[1] + parts[2] + parts[3],
        parts[4] + parts[5] + parts[6] + parts[7],
    ])
    if _trace:
        return out, res
    return out

